# revision 44
# baseline (speedup 1.0000x reference)
"""Masked attention kernel for Trainium2, data-parallel over batch on 8 NeuronCores.

Problem (per reference):
    query (128, 512) f32, key/value (1024, 128, 512) f32, mask (128, 1, 1024) i32
    energy = einsum('bh,tbh->bt'); attn = softmax(energy)
    masked = mask*attn / sum(mask*attn); context = einsum('bt,tbh->bh')
    returns (context (128,512), masked_attention (128,1024))

Key algebraic simplification: the unmasked softmax normalizer cancels:
    masked = m*exp(e-max) / sum(m*exp(e-max))

Per-core structure (B_loc=16, T=1024, H=512; t on partitions, 8 t-tiles):
    - q broadcast on-chip: 2 KB row DMAs + PE outer products (ones^T @ q_row)
    - K and V streamed as (128 t, 8 half-batch, 512 h) tiles: 16 KB contiguous
      bursts at 32 KB stride, 2 MB per DMA
    - energy: fused DVE affine_mul_reduce (K*q_bcast, sum over h) one pass
      -> E_all (128 t-part, tt*16+b columns)
    - PE transposes E to row layout (16 b-part, 1024 t); softmax row-wise:
      reduce_max(negate) -> ACT Exp(bias=-max) -> mask mult + sum -> reciprocal
    - attn rows scaled by 1/Z, DMA'd out; attn transposed back to columns
    - context: all 128 PE matmuls (float32r) accumulate into ONE (16,512)
      psum tile; lhsT is column-masked (only column b nonzero) so each row
      accumulates exactly its own batch. One copy + one 32 KB output DMA.
"""

import numpy as np

B, T, H = 128, 1024, 512
NCORES = 8
BL = B // NCORES  # 16 batches per core
NT = T // 128     # 8 t-tiles
HB = BL // 2      # half-batch chunk (K/V tiles)

_cache = {}


def _build_nc(debug=False):
    from contextlib import ExitStack

    import concourse.bacc as bacc
    import concourse.bass as bass
    import concourse.mybir as mybir
    import concourse.tile as tile
    from concourse import masks

    f32 = mybir.dt.float32
    f32r = mybir.dt.float32r
    i32 = mybir.dt.int32
    Alu = mybir.AluOpType
    Act = mybir.ActivationFunctionType

    nc = bacc.Bacc("TRN2", target_bir_lowering=False, debug=debug)

    q_d = nc.dram_tensor("query", [BL, H], f32, kind="ExternalInput")
    k_d = nc.dram_tensor("key", [T, BL, H], f32, kind="ExternalInput")
    v_d = nc.dram_tensor("value", [T, BL, H], f32r, kind="ExternalInput")
    m_d = nc.dram_tensor("mask", [BL, 1, T], i32, kind="ExternalInput")
    ctx_d = nc.dram_tensor("out_ctx", [BL, H], f32, kind="ExternalOutput")
    attn_d = nc.dram_tensor("out_attn", [BL, T], f32, kind="ExternalOutput")

    # (T, BL, H) -> (NT, 2, 128, HB, H): per (t-tile, half-batch) tiles.
    # Each [tt, hf] slice = 128 partitions x 16 KB contiguous, 32 KB stride.
    k_r = k_d.ap().rearrange("(tt p) (hf b) h -> tt hf p b h", p=128, b=HB)
    v_r = v_d.ap().rearrange("(tt p) (hf b) h -> tt hf p b h", p=128, b=HB)

    with tile.TileContext(nc) as tc, ExitStack() as ctx:
        const = ctx.enter_context(tc.tile_pool(name="const", bufs=1))
        once = ctx.enter_context(tc.tile_pool(name="once", bufs=1))
        kpool = ctx.enter_context(tc.tile_pool(name="kpool", bufs=3))
        vpool = ctx.enter_context(tc.tile_pool(name="vpool", bufs=4))
        prodp = ctx.enter_context(tc.tile_pool(name="prodp", bufs=2))
        psum_e = ctx.enter_context(
            tc.tile_pool(name="psum_e", bufs=1, space=bass.MemorySpace.PSUM)
        )
        psum_s = ctx.enter_context(
            tc.tile_pool(name="psum_s", bufs=2, space=bass.MemorySpace.PSUM)
        )
        lhsp = ctx.enter_context(tc.tile_pool(name="lhsp", bufs=16))
        psum_w = ctx.enter_context(
            tc.tile_pool(name="psum_w", bufs=1, space=bass.MemorySpace.PSUM)
        )
        psum_c = ctx.enter_context(
            tc.tile_pool(name="psum_c", bufs=1, space=bass.MemorySpace.PSUM)
        )

        identity = const.tile([128, 128], f32)
        masks.make_identity(nc, identity[:])

        mask_i = once.tile([BL, T], i32)
        nc.sync.dma_start(mask_i[:], m_d.ap().rearrange("b o t -> b (o t)"))
        mask_f = const.tile([BL, T], f32)
        nc.vector.tensor_copy(mask_f[:], mask_i[:])

        # one-hot column masks for the context accumulation:
        # colmask[:, b*BL + j] = 1.0 iff j == b
        colmask = const.tile([128, BL * BL], f32)
        nc.gpsimd.memset(colmask[:], 0.0)
        for b in range(BL):
            nc.vector.memset(colmask[:, b * BL + b : b * BL + b + 1], 1.0)

        # broadcast each query row to all 128 partitions via 0-stride DRAM
        # reads. Costs 4 MB of redundant HBM traffic, but lands first on the
        # DMA queue with no compute dependencies: a PE-based broadcast stalls
        # the whole K stream ~30 us behind cold fp32 matmuls at startup.
        qb = const.tile([128, BL, H], f32)
        for b in range(BL):
            nc.sync.dma_start(
                qb[:, b, :], q_d.ap()[b : b + 1, :].to_broadcast((128, H))
            )

        # ---- energy: E_all[p, tt*BL+b] = sum_h K[tt*128+p, b, h] * q[b, h]
        # tt-outer so each t-tile's transpose + partial row-max pipeline
        # behind the K stream instead of serializing at the end
        E_all = const.tile([128, NT * BL], f32)
        erow = psum_e.tile([BL, T], f32)
        pmax = const.tile([BL, NT], f32)
        for tt in range(NT):
            for hf in range(2):
                kt = kpool.tile([128, HB, H], f32, tag="kt")
                nc.sync.dma_start(kt[:], k_r[tt, hf])

                # balance the reduction between DVE (fused mult+reduce) and
                # ACT (reduce of a DVE-computed product); GpSimd is useless
                # here — it shares (and exclusively locks) DVE's SBUF port
                for j in range(HB):
                    b = hf * HB + j
                    col = E_all[:, tt * BL + b : tt * BL + b + 1]
                    pr = prodp.tile([128, H], f32, tag="pr")
                    if j < 3:
                        nc.vector.affine_mul_reduce(
                            out=pr[:],
                            accum_out=col,
                            in0=kt[:, j, :],
                            in1=qb[:, b, :],
                            scale=1.0,
                            bias=0.0,
                        )
                    else:
                        nc.vector.tensor_tensor(
                            out=pr[:], in0=kt[:, j, :], in1=qb[:, b, :], op=Alu.mult
                        )
                        pr2 = psum_s.tile([128, H], f32)
                        nc.scalar.activation(
                            pr2[:], pr[:], Act.Identity, accum_out=col
                        )
            # transpose this t-tile to row layout and take its partial max
            nc.tensor.transpose(
                erow[:, tt * 128 : (tt + 1) * 128],
                E_all[:, tt * BL : (tt + 1) * BL],
                identity[:],
            )
            nc.vector.tensor_reduce(
                pmax[:, tt : tt + 1],
                erow[:, tt * 128 : (tt + 1) * 128],
                axis=mybir.AxisListType.X,
                op=Alu.max,
            )

        # ---- softmax (row-wise over free dim)
        negmax = const.tile([BL, 1], f32)
        nc.vector.tensor_reduce(
            negmax[:], pmax[:], axis=mybir.AxisListType.X, op=Alu.max, negate=True
        )
        xrow = once.tile([BL, T], f32)
        nc.scalar.activation(xrow[:], erow[:], Act.Exp, bias=negmax[:], scale=1.0)
        wrow = const.tile([BL, T], f32)
        zsum = const.tile([BL, 1], f32)
        nc.vector.tensor_tensor(out=wrow[:], in0=xrow[:], in1=mask_f[:], op=Alu.mult)
        nc.vector.tensor_reduce(
            zsum[:], wrow[:], axis=mybir.AxisListType.X, op=Alu.add
        )
        rz = const.tile([BL, 1], f32)
        nc.vector.reciprocal(rz[:], zsum[:])
        attn = const.tile([BL, T], f32)
        nc.vector.tensor_scalar_mul(attn[:], wrow[:], rz[:])
        # output DMAs go on the ACT queue: the sync queue is FIFO per engine,
        # and a compute-dependent DMA there would head-of-line block V loads
        nc.scalar.dma_start(attn_d.ap(), attn[:])

        # ---- transpose normalized attn to column layout:
        #      wcol[p, tt*BL+b] = attn[b, tt*128+p]  (already scaled by 1/Z)
        wcol_ps = psum_w.tile([128, NT * BL], f32)
        for tt in range(NT):
            nc.tensor.transpose(
                wcol_ps[:, tt * BL : (tt + 1) * BL],
                attn[:, tt * 128 : (tt + 1) * 128],
                identity[:BL, :BL],
            )
        wcol = const.tile([128, NT * BL], f32)
        nc.scalar.copy(wcol[:], wcol_ps[:])

        # ---- context: ctx[b, h] = sum_t attn[b, t] * V[t, b, h]
        # All 128 float32r matmuls accumulate into one (16,512) psum tile.
        # lhsT for (tt, b) is wcol's tt block masked to column b only, so
        # psum row b accumulates exactly batch b's contributions.
        cps = psum_c.tile([BL, H], f32)
        nmm = NT * BL
        i = 0
        for tt in range(NT):
            for hf in range(2):
                # V loads issue from GpSimd (SWDGE): a separate DMA queue row,
                # so V prefetch fills the gaps whenever the K stream is
                # slot-blocked, without FIFO coupling to the sync queue
                vt = vpool.tile([128, HB, H], f32r, tag="vt")
                nc.gpsimd.dma_start(vt[:], v_r[tt, hf])
                for j in range(HB):
                    b = hf * HB + j
                    lhsT = lhsp.tile([128, BL], f32r, tag="lhsT")
                    nc.vector.tensor_tensor(
                        out=lhsT[:],
                        in0=wcol[:, tt * BL : (tt + 1) * BL],
                        in1=colmask[:, b * BL : (b + 1) * BL],
                        op=Alu.mult,
                    )
                    nc.tensor.matmul(
                        cps[:],
                        lhsT[:],
                        vt[:, j, :],
                        start=(i == 0),
                        stop=(i == nmm - 1),
                    )
                    i += 1
        ctx_sb = const.tile([BL, H], f32)
        nc.scalar.copy(ctx_sb[:], cps[:])
        nc.scalar.dma_start(ctx_d.ap(), ctx_sb[:])

    nc.compile()
    return nc


def _get_nc():
    if "nc" not in _cache:
        _cache["nc"] = _build_nc(debug=False)
    return _cache["nc"]


def _shard_inputs(query, key, value, mask):
    in_maps = []
    for i in range(NCORES):
        s = slice(i * BL, (i + 1) * BL)
        in_maps.append(
            {
                "query": np.ascontiguousarray(query[s]),
                "key": np.ascontiguousarray(key[:, s]),
                "value": np.ascontiguousarray(value[:, s]),
                "mask": np.ascontiguousarray(mask[s]),
            }
        )
    return in_maps


def run_sharded(query, key, value, mask, trace=False, **kw):
    from concourse.bass_utils import run_bass_kernel_spmd

    nc = _get_nc()
    in_maps = _shard_inputs(query, key, value, mask)
    res = run_bass_kernel_spmd(
        nc, in_maps, core_ids=list(range(NCORES)), trace=trace, **kw
    )
    context = np.concatenate([res.results[i]["out_ctx"] for i in range(NCORES)], axis=0)
    attn = np.concatenate([res.results[i]["out_attn"] for i in range(NCORES)], axis=0)
    return (context, attn), res


def kernel(query, key, value, mask):
    query = np.asarray(query, dtype=np.float32)
    key = np.asarray(key, dtype=np.float32)
    value = np.asarray(value, dtype=np.float32)
    mask = np.asarray(mask, dtype=np.int32)
    (context, attn), _ = run_sharded(query, key, value, mask, trace=False)
    return (context, attn)


# revision 47
# speedup vs baseline: 1.0274x; 1.0274x over previous
"""Masked attention kernel for Trainium2, data-parallel over batch on 8 NeuronCores.

Problem (per reference):
    query (128, 512) f32, key/value (1024, 128, 512) f32, mask (128, 1, 1024) i32
    energy = einsum('bh,tbh->bt'); attn = softmax(energy)
    masked = mask*attn / sum(mask*attn); context = einsum('bt,tbh->bh')
    returns (context (128,512), masked_attention (128,1024))

Key algebraic simplification: the unmasked softmax normalizer cancels:
    masked = m*exp(e-max) / sum(m*exp(e-max))

Per-core structure (B_loc=16, T=1024, H=512; t on partitions, 8 t-tiles):
    - q broadcast on-chip: 2 KB row DMAs + PE outer products (ones^T @ q_row)
    - K and V streamed as (128 t, 8 half-batch, 512 h) tiles: 16 KB contiguous
      bursts at 32 KB stride, 2 MB per DMA
    - energy: fused DVE affine_mul_reduce (K*q_bcast, sum over h) one pass
      -> E_all (128 t-part, tt*16+b columns)
    - PE transposes E to row layout (16 b-part, 1024 t); softmax row-wise:
      reduce_max(negate) -> ACT Exp(bias=-max) -> mask mult + sum -> reciprocal
    - attn rows scaled by 1/Z, DMA'd out; attn transposed back to columns
    - context: all 128 PE matmuls (float32r) accumulate into ONE (16,512)
      psum tile; lhsT is column-masked (only column b nonzero) so each row
      accumulates exactly its own batch. One copy + one 32 KB output DMA.
"""

import numpy as np

B, T, H = 128, 1024, 512
NCORES = 8
BL = B // NCORES  # 16 batches per core
NT = T // 128     # 8 t-tiles
HB = BL // 2      # half-batch chunk (K/V tiles)

_cache = {}


def _build_nc(debug=False):
    from contextlib import ExitStack

    import concourse.bacc as bacc
    import concourse.bass as bass
    import concourse.mybir as mybir
    import concourse.tile as tile
    from concourse import masks

    f32 = mybir.dt.float32
    f32r = mybir.dt.float32r
    i32 = mybir.dt.int32
    Alu = mybir.AluOpType
    Act = mybir.ActivationFunctionType

    nc = bacc.Bacc("TRN2", target_bir_lowering=False, debug=debug)

    q_d = nc.dram_tensor("query", [BL, H], f32, kind="ExternalInput")
    k_d = nc.dram_tensor("key", [T, BL, H], f32, kind="ExternalInput")
    v_d = nc.dram_tensor("value", [T, BL, H], f32r, kind="ExternalInput")
    m_d = nc.dram_tensor("mask", [BL, 1, T], i32, kind="ExternalInput")
    ctx_d = nc.dram_tensor("out_ctx", [BL, H], f32, kind="ExternalOutput")
    attn_d = nc.dram_tensor("out_attn", [BL, T], f32, kind="ExternalOutput")

    # (T, BL, H) -> (NT, 2, 128, HB, H): per (t-tile, half-batch) tiles.
    # Each [tt, hf] slice = 128 partitions x 16 KB contiguous, 32 KB stride.
    k_r = k_d.ap().rearrange("(tt p) (hf b) h -> tt hf p b h", p=128, b=HB)
    v_r = v_d.ap().rearrange("(tt p) (hf b) h -> tt hf p b h", p=128, b=HB)

    with tile.TileContext(nc) as tc, ExitStack() as ctx:
        const = ctx.enter_context(tc.tile_pool(name="const", bufs=1))
        once = ctx.enter_context(tc.tile_pool(name="once", bufs=1))
        kpool = ctx.enter_context(tc.tile_pool(name="kpool", bufs=3))
        vpool = ctx.enter_context(tc.tile_pool(name="vpool", bufs=4))
        prodp = ctx.enter_context(tc.tile_pool(name="prodp", bufs=2))
        psum_e = ctx.enter_context(
            tc.tile_pool(name="psum_e", bufs=1, space=bass.MemorySpace.PSUM)
        )
        psum_s = ctx.enter_context(
            tc.tile_pool(name="psum_s", bufs=2, space=bass.MemorySpace.PSUM)
        )
        lhsp = ctx.enter_context(tc.tile_pool(name="lhsp", bufs=16))
        psum_w = ctx.enter_context(
            tc.tile_pool(name="psum_w", bufs=1, space=bass.MemorySpace.PSUM)
        )
        psum_c = ctx.enter_context(
            tc.tile_pool(name="psum_c", bufs=1, space=bass.MemorySpace.PSUM)
        )

        identity = const.tile([128, 128], f32)
        masks.make_identity(nc, identity[:])

        mask_i = once.tile([BL, T], i32)
        nc.sync.dma_start(mask_i[:], m_d.ap().rearrange("b o t -> b (o t)"))
        mask_f = const.tile([BL, T], f32)
        nc.vector.tensor_copy(mask_f[:], mask_i[:])

        # one-hot column masks for the context accumulation:
        # colmask[:, b*BL + j] = 1.0 iff j == b
        colmask = const.tile([128, BL * BL], f32)
        nc.gpsimd.memset(colmask[:], 0.0)
        for b in range(BL):
            nc.vector.memset(colmask[:, b * BL + b : b * BL + b + 1], 1.0)

        # broadcast each query row to all 128 partitions via 0-stride DRAM
        # reads. Costs 4 MB of redundant HBM traffic, but lands first on the
        # DMA queue with no compute dependencies: a PE-based broadcast stalls
        # the whole K stream ~30 us behind cold fp32 matmuls at startup.
        # issued from ACT's queue row: these broadcast reads are slow per
        # byte and must not delay the K stream on the sync queue
        qb = const.tile([128, BL, H], f32)
        for b in range(BL):
            nc.scalar.dma_start(
                qb[:, b, :], q_d.ap()[b : b + 1, :].to_broadcast((128, H))
            )

        # ---- energy: E_all[p, tt*BL+b] = sum_h K[tt*128+p, b, h] * q[b, h]
        # tt-outer so each t-tile's transpose + partial row-max pipeline
        # behind the K stream instead of serializing at the end
        E_all = const.tile([128, NT * BL], f32)
        erow = psum_e.tile([BL, T], f32)
        pmax = const.tile([BL, NT], f32)
        k_gate = None
        for tt in range(NT):
            for hf in range(2):
                kt = kpool.tile([128, HB, H], f32, tag="kt")
                kd = nc.sync.dma_start(kt[:], k_r[tt, hf])
                if tt * 2 + hf == 9:
                    k_gate = kd

                # balance the reduction between DVE (fused mult+reduce) and
                # ACT (reduce of a DVE-computed product); GpSimd is useless
                # here — it shares (and exclusively locks) DVE's SBUF port
                for j in range(HB):
                    b = hf * HB + j
                    col = E_all[:, tt * BL + b : tt * BL + b + 1]
                    pr = prodp.tile([128, H], f32, tag="pr")
                    if j < 3:
                        nc.vector.affine_mul_reduce(
                            out=pr[:],
                            accum_out=col,
                            in0=kt[:, j, :],
                            in1=qb[:, b, :],
                            scale=1.0,
                            bias=0.0,
                        )
                    else:
                        nc.vector.tensor_tensor(
                            out=pr[:], in0=kt[:, j, :], in1=qb[:, b, :], op=Alu.mult
                        )
                        pr2 = psum_s.tile([128, H], f32)
                        nc.scalar.activation(
                            pr2[:], pr[:], Act.Identity, accum_out=col
                        )
            # transpose this t-tile to row layout and take its partial max
            nc.tensor.transpose(
                erow[:, tt * 128 : (tt + 1) * 128],
                E_all[:, tt * BL : (tt + 1) * BL],
                identity[:],
            )
            nc.vector.tensor_reduce(
                pmax[:, tt : tt + 1],
                erow[:, tt * 128 : (tt + 1) * 128],
                axis=mybir.AxisListType.X,
                op=Alu.max,
            )

        # ---- softmax (row-wise over free dim)
        negmax = const.tile([BL, 1], f32)
        nc.vector.tensor_reduce(
            negmax[:], pmax[:], axis=mybir.AxisListType.X, op=Alu.max, negate=True
        )
        xrow = once.tile([BL, T], f32)
        nc.scalar.activation(xrow[:], erow[:], Act.Exp, bias=negmax[:], scale=1.0)
        wrow = const.tile([BL, T], f32)
        zsum = const.tile([BL, 1], f32)
        nc.vector.tensor_tensor(out=wrow[:], in0=xrow[:], in1=mask_f[:], op=Alu.mult)
        nc.vector.tensor_reduce(
            zsum[:], wrow[:], axis=mybir.AxisListType.X, op=Alu.add
        )
        rz = const.tile([BL, 1], f32)
        nc.vector.reciprocal(rz[:], zsum[:])
        attn = const.tile([BL, T], f32)
        nc.vector.tensor_scalar_mul(attn[:], wrow[:], rz[:])
        # output DMAs go on the ACT queue: the sync queue is FIFO per engine,
        # and a compute-dependent DMA there would head-of-line block V loads
        nc.scalar.dma_start(attn_d.ap(), attn[:])

        # ---- transpose normalized attn to column layout:
        #      wcol[p, tt*BL+b] = attn[b, tt*128+p]  (already scaled by 1/Z)
        wcol_ps = psum_w.tile([128, NT * BL], f32)
        for tt in range(NT):
            nc.tensor.transpose(
                wcol_ps[:, tt * BL : (tt + 1) * BL],
                attn[:, tt * 128 : (tt + 1) * 128],
                identity[:BL, :BL],
            )
        wcol = const.tile([128, NT * BL], f32)
        nc.scalar.copy(wcol[:], wcol_ps[:])

        # ---- context: ctx[b, h] = sum_t attn[b, t] * V[t, b, h]
        # All 128 float32r matmuls accumulate into one (16,512) psum tile.
        # lhsT for (tt, b) is wcol's tt block masked to column b only, so
        # psum row b accumulates exactly batch b's contributions.
        cps = psum_c.tile([BL, H], f32)
        nmm = NT * BL
        i = 0
        for tt in range(NT):
            for hf in range(2):
                # V loads issue from GpSimd (SWDGE): a separate DMA queue row,
                # so V prefetch fills the gaps whenever the K stream is
                # slot-blocked, without FIFO coupling to the sync queue.
                # The first chunks are gated on K tile #10 so the prefetch
                # lands at the softmax boundary instead of starving early K.
                from concourse.tile import add_dep_helper

                vt = vpool.tile([128, HB, H], f32r, tag="vt")
                vd = nc.gpsimd.dma_start(vt[:], v_r[tt, hf])
                if tt * 2 + hf < 4 and k_gate is not None:
                    add_dep_helper(
                        vd.ins, k_gate.ins, sync=True,
                        reason="gate V prefetch behind late K stream",
                    )
                for j in range(HB):
                    b = hf * HB + j
                    lhsT = lhsp.tile([128, BL], f32r, tag="lhsT")
                    nc.vector.tensor_tensor(
                        out=lhsT[:],
                        in0=wcol[:, tt * BL : (tt + 1) * BL],
                        in1=colmask[:, b * BL : (b + 1) * BL],
                        op=Alu.mult,
                    )
                    nc.tensor.matmul(
                        cps[:],
                        lhsT[:],
                        vt[:, j, :],
                        start=(i == 0),
                        stop=(i == nmm - 1),
                    )
                    i += 1
        ctx_sb = const.tile([BL, H], f32)
        nc.scalar.copy(ctx_sb[:], cps[:])
        nc.scalar.dma_start(ctx_d.ap(), ctx_sb[:])

    nc.compile()
    return nc


def _get_nc():
    if "nc" not in _cache:
        _cache["nc"] = _build_nc(debug=False)
    return _cache["nc"]


def _shard_inputs(query, key, value, mask):
    in_maps = []
    for i in range(NCORES):
        s = slice(i * BL, (i + 1) * BL)
        in_maps.append(
            {
                "query": np.ascontiguousarray(query[s]),
                "key": np.ascontiguousarray(key[:, s]),
                "value": np.ascontiguousarray(value[:, s]),
                "mask": np.ascontiguousarray(mask[s]),
            }
        )
    return in_maps


def run_sharded(query, key, value, mask, trace=False, **kw):
    from concourse.bass_utils import run_bass_kernel_spmd

    nc = _get_nc()
    in_maps = _shard_inputs(query, key, value, mask)
    res = run_bass_kernel_spmd(
        nc, in_maps, core_ids=list(range(NCORES)), trace=trace, **kw
    )
    context = np.concatenate([res.results[i]["out_ctx"] for i in range(NCORES)], axis=0)
    attn = np.concatenate([res.results[i]["out_attn"] for i in range(NCORES)], axis=0)
    return (context, attn), res


def kernel(query, key, value, mask):
    query = np.asarray(query, dtype=np.float32)
    key = np.asarray(key, dtype=np.float32)
    value = np.asarray(value, dtype=np.float32)
    mask = np.asarray(mask, dtype=np.int32)
    (context, attn), _ = run_sharded(query, key, value, mask, trace=False)
    return (context, attn)


# revision 51
# speedup vs baseline: 1.0342x; 1.0066x over previous
"""Masked attention kernel for Trainium2, data-parallel over batch on 8 NeuronCores.

Problem (per reference):
    query (128, 512) f32, key/value (1024, 128, 512) f32, mask (128, 1, 1024) i32
    energy = einsum('bh,tbh->bt'); attn = softmax(energy)
    masked = mask*attn / sum(mask*attn); context = einsum('bt,tbh->bh')
    returns (context (128,512), masked_attention (128,1024))

Key algebraic simplification: the unmasked softmax normalizer cancels:
    masked = m*exp(e-max) / sum(m*exp(e-max))

Per-core structure (B_loc=16, T=1024, H=512; t on partitions, 8 t-tiles):
    - q broadcast on-chip: 2 KB row DMAs + PE outer products (ones^T @ q_row)
    - K and V streamed as (128 t, 8 half-batch, 512 h) tiles: 16 KB contiguous
      bursts at 32 KB stride, 2 MB per DMA
    - energy: fused DVE affine_mul_reduce (K*q_bcast, sum over h) one pass
      -> E_all (128 t-part, tt*16+b columns)
    - PE transposes E to row layout (16 b-part, 1024 t); softmax row-wise:
      reduce_max(negate) -> ACT Exp(bias=-max) -> mask mult + sum -> reciprocal
    - attn rows scaled by 1/Z, DMA'd out; attn transposed back to columns
    - context: all 128 PE matmuls (float32r) accumulate into ONE (16,512)
      psum tile; lhsT is column-masked (only column b nonzero) so each row
      accumulates exactly its own batch. One copy + one 32 KB output DMA.
"""

import numpy as np

B, T, H = 128, 1024, 512
NCORES = 8
BL = B // NCORES  # 16 batches per core
NT = T // 128     # 8 t-tiles
HB = BL // 2      # half-batch chunk (K/V tiles)

_cache = {}


def _build_nc(debug=False):
    from contextlib import ExitStack

    import concourse.bacc as bacc
    import concourse.bass as bass
    import concourse.mybir as mybir
    import concourse.tile as tile
    from concourse import masks

    f32 = mybir.dt.float32
    f32r = mybir.dt.float32r
    i32 = mybir.dt.int32
    Alu = mybir.AluOpType
    Act = mybir.ActivationFunctionType

    nc = bacc.Bacc("TRN2", target_bir_lowering=False, debug=debug)

    q_d = nc.dram_tensor("query", [BL, H], f32, kind="ExternalInput")
    k_d = nc.dram_tensor("key", [T, BL, H], f32, kind="ExternalInput")
    v_d = nc.dram_tensor("value", [T, BL, H], f32r, kind="ExternalInput")
    m_d = nc.dram_tensor("mask", [BL, 1, T], i32, kind="ExternalInput")
    ctx_d = nc.dram_tensor("out_ctx", [BL, H], f32, kind="ExternalOutput")
    attn_d = nc.dram_tensor("out_attn", [BL, T], f32, kind="ExternalOutput")

    # (T, BL, H) -> (NT, 2, 128, HB, H): per (t-tile, half-batch) tiles.
    # Each [tt, hf] slice = 128 partitions x 16 KB contiguous, 32 KB stride.
    k_r = k_d.ap().rearrange("(tt p) (hf b) h -> tt hf p b h", p=128, b=HB)
    # V in quarter-batch chunks (1 MB DMAs): faster slot turnover around
    # the softmax boundary
    QB = BL // 4
    v_r = v_d.ap().rearrange("(tt p) (qf b) h -> tt qf p b h", p=128, b=QB)

    with tile.TileContext(nc) as tc, ExitStack() as ctx:
        const = ctx.enter_context(tc.tile_pool(name="const", bufs=1))
        once = ctx.enter_context(tc.tile_pool(name="once", bufs=1))
        kpool = ctx.enter_context(tc.tile_pool(name="kpool", bufs=3))
        vpool = ctx.enter_context(tc.tile_pool(name="vpool", bufs=8))
        prodp = ctx.enter_context(tc.tile_pool(name="prodp", bufs=2))
        psum_e = ctx.enter_context(
            tc.tile_pool(name="psum_e", bufs=1, space=bass.MemorySpace.PSUM)
        )
        psum_s = ctx.enter_context(
            tc.tile_pool(name="psum_s", bufs=2, space=bass.MemorySpace.PSUM)
        )
        lhsp = ctx.enter_context(tc.tile_pool(name="lhsp", bufs=16))
        psum_w = ctx.enter_context(
            tc.tile_pool(name="psum_w", bufs=1, space=bass.MemorySpace.PSUM)
        )
        psum_c = ctx.enter_context(
            tc.tile_pool(name="psum_c", bufs=1, space=bass.MemorySpace.PSUM)
        )

        identity = const.tile([128, 128], f32)
        masks.make_identity(nc, identity[:])

        mask_i = once.tile([BL, T], i32)
        nc.sync.dma_start(mask_i[:], m_d.ap().rearrange("b o t -> b (o t)"))
        mask_f = const.tile([BL, T], f32)
        nc.vector.tensor_copy(mask_f[:], mask_i[:])

        # one-hot column masks for the context accumulation:
        # colmask[:, b*BL + j] = 1.0 iff j == b
        colmask = const.tile([128, BL * BL], f32)
        nc.gpsimd.memset(colmask[:], 0.0)
        for b in range(BL):
            nc.vector.memset(colmask[:, b * BL + b : b * BL + b + 1], 1.0)

        # broadcast each query row to all 128 partitions via 0-stride DRAM
        # reads. Costs 4 MB of redundant HBM traffic, but lands first on the
        # DMA queue with no compute dependencies: a PE-based broadcast stalls
        # the whole K stream ~30 us behind cold fp32 matmuls at startup.
        # issued from ACT's queue row: these broadcast reads are slow per
        # byte and must not delay the K stream on the sync queue
        qb = const.tile([128, BL, H], f32)
        for b in range(BL):
            nc.scalar.dma_start(
                qb[:, b, :], q_d.ap()[b : b + 1, :].to_broadcast((128, H))
            )

        # ---- energy: E_all[p, tt*BL+b] = sum_h K[tt*128+p, b, h] * q[b, h]
        # tt-outer so each t-tile's transpose + partial row-max pipeline
        # behind the K stream instead of serializing at the end
        E_all = const.tile([128, NT * BL], f32)
        erow = psum_e.tile([BL, T], f32)
        pmax = const.tile([BL, NT], f32)
        k_gate = None
        for tt in range(NT):
            for hf in range(2):
                kt = kpool.tile([128, HB, H], f32, tag="kt")
                kd = nc.sync.dma_start(kt[:], k_r[tt, hf])
                if tt * 2 + hf == 9:
                    k_gate = kd

                # balance the reduction between DVE (fused mult+reduce) and
                # ACT (reduce of a DVE-computed product); GpSimd is useless
                # here — it shares (and exclusively locks) DVE's SBUF port
                for j in range(HB):
                    b = hf * HB + j
                    col = E_all[:, tt * BL + b : tt * BL + b + 1]
                    pr = prodp.tile([128, H], f32, tag="pr")
                    if j < 3:
                        nc.vector.affine_mul_reduce(
                            out=pr[:],
                            accum_out=col,
                            in0=kt[:, j, :],
                            in1=qb[:, b, :],
                            scale=1.0,
                            bias=0.0,
                        )
                    else:
                        nc.vector.tensor_tensor(
                            out=pr[:], in0=kt[:, j, :], in1=qb[:, b, :], op=Alu.mult
                        )
                        pr2 = psum_s.tile([128, H], f32)
                        nc.scalar.activation(
                            pr2[:], pr[:], Act.Identity, accum_out=col
                        )
            # transpose this t-tile to row layout and take its partial max
            nc.tensor.transpose(
                erow[:, tt * 128 : (tt + 1) * 128],
                E_all[:, tt * BL : (tt + 1) * BL],
                identity[:],
            )
            nc.vector.tensor_reduce(
                pmax[:, tt : tt + 1],
                erow[:, tt * 128 : (tt + 1) * 128],
                axis=mybir.AxisListType.X,
                op=Alu.max,
            )

        # ---- softmax (row-wise over free dim)
        negmax = const.tile([BL, 1], f32)
        nc.vector.tensor_reduce(
            negmax[:], pmax[:], axis=mybir.AxisListType.X, op=Alu.max, negate=True
        )
        xrow = once.tile([BL, T], f32)
        nc.scalar.activation(xrow[:], erow[:], Act.Exp, bias=negmax[:], scale=1.0)
        # fused: wrow = xrow * mask, zsum = row-sum(wrow)
        wrow = const.tile([BL, T], f32)
        zsum = const.tile([BL, 1], f32)
        nc.vector.affine_mul_reduce(
            out=wrow[:], accum_out=zsum[:], in0=xrow[:], in1=mask_f[:],
            scale=1.0, bias=0.0,
        )
        rz = const.tile([BL, 1], f32)
        nc.vector.reciprocal(rz[:], zsum[:])

        # transpose UNNORMALIZED weights — the 1/Z scale is applied to the
        # attn output (off the critical path) and folded into the context
        # epilogue copy, so the matmuls start as soon as wrow exists
        wcol_ps = psum_w.tile([128, NT * BL], f32)
        for tt in range(NT):
            nc.tensor.transpose(
                wcol_ps[:, tt * BL : (tt + 1) * BL],
                wrow[:, tt * 128 : (tt + 1) * 128],
                identity[:BL, :BL],
            )
        wcol = const.tile([128, NT * BL], f32)
        nc.scalar.copy(wcol[:], wcol_ps[:])

        attn = const.tile([BL, T], f32)
        nc.vector.tensor_scalar_mul(attn[:], wrow[:], rz[:])
        # output DMAs go on the ACT queue: the sync queue is FIFO per engine,
        # and a compute-dependent DMA there would head-of-line block V loads
        nc.scalar.dma_start(attn_d.ap(), attn[:])

        # ---- context: ctx[b, h] = sum_t attn[b, t] * V[t, b, h]
        # All 128 float32r matmuls accumulate into one (16,512) psum tile.
        # lhsT for (tt, b) is wcol's tt block masked to column b only, so
        # psum row b accumulates exactly batch b's contributions.
        from concourse.tile import add_dep_helper

        cps = psum_c.tile([BL, H], f32)
        nmm = NT * BL
        i = 0
        for tt in range(NT):
            for qf in range(4):
                # V loads issue from GpSimd (SWDGE): a separate DMA queue row,
                # so V prefetch fills the gaps whenever the K stream is
                # slot-blocked, without FIFO coupling to the sync queue.
                # The first chunks are gated on K tile #10 so the prefetch
                # lands at the softmax boundary instead of starving early K.
                vt = vpool.tile([128, QB, H], f32r, tag="vt")
                vd = nc.gpsimd.dma_start(vt[:], v_r[tt, qf])
                if tt * 4 + qf < 8 and k_gate is not None:
                    add_dep_helper(
                        vd.ins, k_gate.ins, sync=True,
                        reason="gate V prefetch behind late K stream",
                    )
                for j in range(QB):
                    b = qf * QB + j
                    lhsT = lhsp.tile([128, BL], f32r, tag="lhsT")
                    nc.vector.tensor_tensor(
                        out=lhsT[:],
                        in0=wcol[:, tt * BL : (tt + 1) * BL],
                        in1=colmask[:, b * BL : (b + 1) * BL],
                        op=Alu.mult,
                    )
                    nc.tensor.matmul(
                        cps[:],
                        lhsT[:],
                        vt[:, j, :],
                        start=(i == 0),
                        stop=(i == nmm - 1),
                    )
                    i += 1
        # epilogue: one copy with the 1/Z row scale folded in, then DMA out
        ctx_sb = const.tile([BL, H], f32)
        nc.scalar.activation(ctx_sb[:], cps[:], Act.Copy, scale=rz[:])
        nc.scalar.dma_start(ctx_d.ap(), ctx_sb[:])

    nc.compile()
    return nc


def _get_nc():
    if "nc" not in _cache:
        _cache["nc"] = _build_nc(debug=False)
    return _cache["nc"]


def _shard_inputs(query, key, value, mask):
    in_maps = []
    for i in range(NCORES):
        s = slice(i * BL, (i + 1) * BL)
        in_maps.append(
            {
                "query": np.ascontiguousarray(query[s]),
                "key": np.ascontiguousarray(key[:, s]),
                "value": np.ascontiguousarray(value[:, s]),
                "mask": np.ascontiguousarray(mask[s]),
            }
        )
    return in_maps


def run_sharded(query, key, value, mask, trace=False, **kw):
    from concourse.bass_utils import run_bass_kernel_spmd

    nc = _get_nc()
    in_maps = _shard_inputs(query, key, value, mask)
    res = run_bass_kernel_spmd(
        nc, in_maps, core_ids=list(range(NCORES)), trace=trace, **kw
    )
    context = np.concatenate([res.results[i]["out_ctx"] for i in range(NCORES)], axis=0)
    attn = np.concatenate([res.results[i]["out_attn"] for i in range(NCORES)], axis=0)
    return (context, attn), res


def kernel(query, key, value, mask):
    query = np.asarray(query, dtype=np.float32)
    key = np.asarray(key, dtype=np.float32)
    value = np.asarray(value, dtype=np.float32)
    mask = np.asarray(mask, dtype=np.int32)
    (context, attn), _ = run_sharded(query, key, value, mask, trace=False)
    return (context, attn)
